# revision 27
# baseline (speedup 1.0000x reference)
"""Trainium2 Bass kernel for the double-softmax 4D MultiHeadAttention.

Problem shapes (hardcoded): B=2, L=64, F=32, C=64, H=8.
Per (b,h) head this is a 2048x2048 attention over (l,f)x(m,g) with channel
contraction C=64, a softmax over m (key seq) then over g (key feature),
then a value einsum and output projection.

Sharding: 16 (b,h) pairs over 8 cores -> 2 heads per core (core i handles
b = i//4, heads {2*(i%4), 2*(i%4)+1}).  Each core computes the partial
output projection sum over its heads; host sums the 4 partials per b and
adds the output bias.

Device algorithm per head (all in "T layout": partition = key index kappa,
free = query index lf), with keys permuted to (g,m) order (kappa = g*64+m):
  A: S^T = K' Q^T / 8 (scale folded into Wq), E1 = exp(S^T)        [PE+ACT]
  B: colsum over m  via 0/1 selection matmuls -> recip1            [PE+DVE]
  C: W1 = E1 * recip1[g(p)] (rep via PE matmul), E2 = exp(W1)      [PE+DVE+ACT]
  D: rowsum over g  via selection matmuls -> recip2 (tile-invariant
     because m = p mod 64 in (g,m) order) -> replicate via DMA     [PE+DVE+DMA]
  E: W2 = E2 * recip2[m(p)], Z^T accumulated = V'^T @ W2           [DVE+PE]
  F: outT += Wo_h^T @ Z^T                                          [PE]
"""

import numpy as np
import ml_dtypes

import concourse.bass as bass
import concourse.tile as tile
from concourse import bacc, mybir
from concourse.bass_utils import run_bass_kernel_spmd

B, L, F, C, H = 2, 64, 32, 64, 8
LF = L * F            # 2048
NCORES = 8
HEADS_PER_CORE = 2
NTILES = LF // 128    # 16 key-index chunks
FP32 = mybir.dt.float32
BF16 = mybir.dt.bfloat16

_g_idx = np.arange(LF) // 64
_m_idx = np.arange(LF) % 64
_PERM_GM = _m_idx * 32 + _g_idx   # (g,m)-ordered position -> natural key index


def _build_program():
    nc = bacc.Bacc(None, target_bir_lowering=False)

    # --- DRAM I/O (per-core tensors; same program on all cores) ---
    d_xq = nc.dram_tensor("xq", [C + 1, LF], BF16, kind="ExternalInput")
    d_xk = nc.dram_tensor("xk", [C + 1, LF], BF16, kind="ExternalInput")   # gm-permuted
    d_xv = nc.dram_tensor("xv", [C + 1, LF], BF16, kind="ExternalInput")   # gm-permuted
    d_wq = nc.dram_tensor("wq", [C + 1, 128], BF16, kind="ExternalInput")  # 2 heads
    d_wk = nc.dram_tensor("wk", [C + 1, 128], BF16, kind="ExternalInput")
    d_wv = nc.dram_tensor("wv", [C + 1, 128], BF16, kind="ExternalInput")
    d_wo = nc.dram_tensor("wo", [C, 128], BF16, kind="ExternalInput")
    d_sel1 = nc.dram_tensor("sel1", [128, NTILES * 32], BF16, kind="ExternalInput")
    d_sel2 = nc.dram_tensor("sel2", [128, 64], BF16, kind="ExternalInput")
    d_repg = nc.dram_tensor("repg", [64, NTILES * 128], BF16, kind="ExternalInput")
    d_out = nc.dram_tensor("outT", [C, LF], FP32, kind="ExternalOutput")

    with tile.TileContext(nc) as tc:
        with (
            tc.tile_pool(name="singles", bufs=1) as singles,
            tc.tile_pool(name="epool", bufs=20) as epool,
            tc.tile_pool(name="w1p", bufs=3) as w1p,
            tc.tile_pool(name="w2p", bufs=3) as w2p,
            tc.tile_pool(name="ps_work", bufs=2, space="PSUM") as ps_work,
            tc.tile_pool(name="ps_red", bufs=1, space="PSUM") as ps_red,
            tc.tile_pool(name="ps_z", bufs=1, space="PSUM") as ps_z,
        ):
            # ---- load inputs ----
            # xq/xk/xv share one staging buffer (used sequentially by the
            # projections) to save SBUF.
            t_wq = singles.tile([C + 1, 128], BF16, tag="wq")
            t_wk = singles.tile([C + 1, 128], BF16, tag="wk")
            t_wv = singles.tile([C + 1, 128], BF16, tag="wv")
            t_wo = singles.tile([C, 128], BF16, tag="wo")
            t_sel1 = singles.tile([128, NTILES * 32], BF16, tag="sel1")
            t_sel2 = singles.tile([128, 64], BF16, tag="sel2")
            t_repg = singles.tile([64, NTILES * 128], BF16, tag="repg")
            for t, d in [(t_wq, d_wq), (t_wk, d_wk), (t_wv, d_wv),
                         (t_wo, d_wo), (t_sel1, d_sel1), (t_sel2, d_sel2),
                         (t_repg, d_repg)]:
                nc.sync.dma_start(out=t[:], in_=d[:])

            # ---- P: projections (both heads at once) ----
            # QT/KT: [128 = 2 heads x C, LF]; V': [128 kappa, 16, 128 = 2 heads x C]
            t_qt = singles.tile([128, LF], BF16, tag="qt")
            t_kt = singles.tile([128, LF], BF16, tag="kt")
            t_v = singles.tile([128, NTILES, 128], BF16, tag="v")
            xpool = tc.alloc_tile_pool(name="xstage", bufs=2)
            for d_x, dst in [(d_xq, "q"), (d_xk, "k"), (d_xv, "v")]:
                t_x = xpool.tile([C + 1, LF], BF16, tag="x", name="xstage")
                nc.sync.dma_start(out=t_x[:], in_=d_x[:])
                if dst == "q":
                    for j in range(2):
                        ps = ps_work.tile([128, 1024], FP32, tag="work")
                        for jj in range(2):
                            sl = slice(j * 1024 + jj * 512,
                                       j * 1024 + (jj + 1) * 512)
                            nc.tensor.matmul(ps[:, jj * 512:(jj + 1) * 512],
                                             t_wq[:], t_x[:, sl],
                                             start=True, stop=True)
                        nc.scalar.copy(t_qt[:, j * 1024:(j + 1) * 1024], ps[:])
                elif dst == "k":
                    for j in range(2):
                        ps = ps_work.tile([128, 1024], FP32, tag="work")
                        for jj in range(2):
                            sl = slice(j * 1024 + jj * 512,
                                       j * 1024 + (jj + 1) * 512)
                            nc.tensor.matmul(ps[:, jj * 512:(jj + 1) * 512],
                                             t_wk[:], t_x[:, sl],
                                             start=True, stop=True)
                        nc.scalar.copy(t_kt[:, j * 1024:(j + 1) * 1024], ps[:])
                else:
                    for j in range(2):
                        ps = ps_work.tile([128, 1024], FP32, tag="work")
                        for jj in range(8):
                            r = j * 8 + jj
                            nc.tensor.matmul(ps[:, jj * 128:(jj + 1) * 128],
                                             t_x[:, r * 128:(r + 1) * 128],
                                             t_wv[:], start=True, stop=True)
                        nc.vector.tensor_copy(t_v[:, j * 8:(j + 1) * 8, :], ps[:])
            xpool.release()

            # persistent per-head / per-core tiles
            t_r1 = singles.tile([32, LF], FP32, tag="r1")       # recip1 fp32
            t_r1b = singles.tile([64, LF], BF16, tag="r1b")     # recip1 bf16 x2 rows
            t_r2f = singles.tile([C, 1024], FP32, tag="r2f")    # recip2 scratch
            t_r2 = singles.tile([128, LF], BF16, tag="r2")      # recip2 replicated
            t_z = [singles.tile([C, LF], BF16, tag=f"z{h}", name=f"z{h}")
                   for h in range(2)]

            for h in range(HEADS_PER_CORE):
                hp = slice(64 * h, 64 * h + 64)   # head slice in QT/KT partitions
                # ---- A: scores + exp1 ----
                e_tiles = []
                for r in range(NTILES):
                    et = epool.tile([128, LF], BF16, tag="e")
                    e_tiles.append(et)
                    for half in range(2):
                        ps = ps_work.tile([128, 1024], FP32, tag="work")
                        for jj in range(2):
                            sl = slice(half * 1024 + jj * 512,
                                       half * 1024 + (jj + 1) * 512)
                            nc.tensor.matmul(ps[:, jj * 512:(jj + 1) * 512],
                                             t_kt[hp, r * 128:(r + 1) * 128],
                                             t_qt[hp, sl], start=True, stop=True)
                        nc.scalar.activation(et[:, half * 1024:(half + 1) * 1024],
                                             ps[:],
                                             mybir.ActivationFunctionType.Exp)
                # ---- B: colsum over m -> recip1 ----
                for half in range(2):
                    cs = ps_red.tile([64, 1024], FP32, tag="red")
                    for r in range(NTILES):
                        for jj in range(2):
                            sl = slice(half * 1024 + jj * 512,
                                       half * 1024 + (jj + 1) * 512)
                            nc.tensor.matmul(cs[0:32, jj * 512:(jj + 1) * 512],
                                             t_sel1[:, r * 32:(r + 1) * 32],
                                             e_tiles[r][:, sl],
                                             start=(r == 0), stop=(r == NTILES - 1))
                    nc.vector.reciprocal_approx_fast(
                        t_r1[:, half * 1024:(half + 1) * 1024], cs[0:32, :])
                    nc.vector.tensor_copy(t_r1b[0:32, half * 1024:(half + 1) * 1024],
                                          t_r1[:, half * 1024:(half + 1) * 1024])
                    nc.sync.dma_start(
                        out=t_r1b[32:64, half * 1024:(half + 1) * 1024],
                        in_=t_r1b[0:32, half * 1024:(half + 1) * 1024])
                # ---- C: W1 = E1 * rep(recip1), E2 = exp(W1) in place ----
                for r in range(NTILES):
                    for half in range(2):
                        rep = ps_work.tile([128, 1024], FP32, tag="work",
                                           name="rep")
                        for jj in range(2):
                            sl = slice(half * 1024 + jj * 512,
                                       half * 1024 + (jj + 1) * 512)
                            nc.tensor.matmul(rep[:, jj * 512:(jj + 1) * 512],
                                             t_repg[0:32,
                                                    r * 128:(r + 1) * 128],
                                             t_r1b[0:32, sl],
                                             start=True, stop=True)
                        w1 = w1p.tile([128, 1024], FP32, tag="w1")
                        nc.vector.tensor_mul(
                            w1[:], e_tiles[r][:, half * 1024:(half + 1) * 1024],
                            rep[:])
                        nc.scalar.activation(
                            e_tiles[r][:, half * 1024:(half + 1) * 1024],
                            w1[:], mybir.ActivationFunctionType.Exp)
                # ---- D: rowsum over g -> recip2 -> replicate ----
                for half in range(2):
                    rs = ps_red.tile([64, 1024], FP32, tag="red")
                    for r in range(NTILES):
                        for jj in range(2):
                            sl = slice(half * 1024 + jj * 512,
                                       half * 1024 + (jj + 1) * 512)
                            nc.tensor.matmul(rs[:, jj * 512:(jj + 1) * 512],
                                             t_sel2[:], e_tiles[r][:, sl],
                                             start=(r == 0), stop=(r == NTILES - 1))
                    nc.vector.reciprocal_approx_fast(t_r2f[:], rs[:])
                    nc.vector.tensor_copy(t_r2[0:64, half * 1024:(half + 1) * 1024],
                                          t_r2f[:])
                    nc.sync.dma_start(
                        out=t_r2[64:128, half * 1024:(half + 1) * 1024],
                        in_=t_r2[0:64, half * 1024:(half + 1) * 1024])
                # ---- E: W2 = E2 * rep(recip2), Z^T += V'^T @ W2 ----
                for half in range(2):
                    zp = ps_z.tile([64, 1024], FP32, tag="z")
                    for r in range(NTILES):
                        w2 = w2p.tile([128, 1024], BF16, tag="w2")
                        nc.vector.tensor_mul(
                            w2[:], e_tiles[r][:, half * 1024:(half + 1) * 1024],
                            t_r2[:, half * 1024:(half + 1) * 1024])
                        for jj in range(2):
                            nc.tensor.matmul(zp[:, jj * 512:(jj + 1) * 512],
                                             t_v[:, r, 64 * h:64 * h + 64],
                                             w2[:, jj * 512:(jj + 1) * 512],
                                             start=(r == 0), stop=(r == NTILES - 1))
                    nc.scalar.copy(t_z[h][:, half * 1024:(half + 1) * 1024], zp[:])

            # ---- F: output projection, accumulate both heads ----
            t_out = singles.tile([C, LF], FP32, tag="out")
            for half in range(2):
                op = ps_red.tile([64, 1024], FP32, tag="red")
                for h in range(HEADS_PER_CORE):
                    for jj in range(2):
                        sl = slice(half * 1024 + jj * 512,
                                   half * 1024 + (jj + 1) * 512)
                        nc.tensor.matmul(op[:, jj * 512:(jj + 1) * 512],
                                         t_wo[:, 64 * h:64 * h + 64],
                                         t_z[h][:, sl],
                                         start=(h == 0), stop=(h == 1))
                nc.scalar.copy(t_out[:, half * 1024:(half + 1) * 1024], op[:])
            nc.sync.dma_start(out=d_out[:], in_=t_out[:])

    nc.finalize()
    return nc


_PROGRAM = None


def _get_program():
    global _PROGRAM
    if _PROGRAM is None:
        _PROGRAM = _build_program()
    return _PROGRAM


def _make_in_maps(query, key, value, Wq, bq, Wk, bk, Wv, bv, Wo, bo):
    ones = np.ones((1, LF), np.float32)
    # constants shared by all cores
    sel1 = np.zeros((128, NTILES * 32), np.float32)
    for r in range(NTILES):
        for p in range(128):
            sel1[p, r * 32 + (2 * r + p // 64)] = 1.0
    sel2 = np.zeros((128, 64), np.float32)
    for p in range(128):
        sel2[p, p % 64] = 1.0
    repg = np.zeros((64, NTILES * 128), np.float32)
    for r in range(NTILES):
        for p in range(128):
            for k in range(2):
                repg[32 * k + 2 * r + p // 64, r * 128 + p] = 1.0
    sel1 = sel1.astype(ml_dtypes.bfloat16)
    sel2 = sel2.astype(ml_dtypes.bfloat16)
    repg = repg.astype(ml_dtypes.bfloat16)

    in_maps = []
    for core in range(NCORES):
        b = core // 4
        h1 = 2 * (core % 4)
        heads = [h1, h1 + 1]
        Xq = query[b].reshape(LF, C).astype(np.float32)
        Xk = key[b].reshape(LF, C).astype(np.float32)
        Xv = value[b].reshape(LF, C).astype(np.float32)
        xq = np.ascontiguousarray(np.vstack([Xq.T, ones]))
        xk = np.ascontiguousarray(np.vstack([Xk.T, ones])[:, _PERM_GM])
        xv = np.ascontiguousarray(np.vstack([Xv.T, ones])[:, _PERM_GM])
        wq = np.concatenate(
            [np.vstack([Wq[:, h::H], bq[None, h::H]]) / 8.0 for h in heads],
            axis=1).astype(np.float32)
        wk = np.concatenate(
            [np.vstack([Wk[:, h::H], bk[None, h::H]]) for h in heads],
            axis=1).astype(np.float32)
        wv = np.concatenate(
            [np.vstack([Wv[:, h::H], bv[None, h::H]]) for h in heads],
            axis=1).astype(np.float32)
        wo = np.concatenate([Wo[h::H, :] for h in heads], axis=1).astype(np.float32)
        bf = ml_dtypes.bfloat16
        in_maps.append({
            "xq": np.ascontiguousarray(xq).astype(bf),
            "xk": np.ascontiguousarray(xk).astype(bf),
            "xv": np.ascontiguousarray(xv).astype(bf),
            "wq": np.ascontiguousarray(wq).astype(bf),
            "wk": np.ascontiguousarray(wk).astype(bf),
            "wv": np.ascontiguousarray(wv).astype(bf),
            "wo": np.ascontiguousarray(wo).astype(bf),
            "sel1": sel1, "sel2": sel2, "repg": repg,
        })
    return in_maps


def kernel(query, key, value, Wq, bq, Wk, bk, Wv, bv, Wo, bo, _collect=None):
    nc = _get_program()
    in_maps = _make_in_maps(query, key, value, Wq, bq, Wk, bk, Wv, bv, Wo, bo)
    res = run_bass_kernel_spmd(nc, in_maps, list(range(NCORES)),
                               **(_collect or {}))
    if _collect is not None:
        _collect["results"] = res
    out = np.zeros((B, LF, C), np.float32)
    for core in range(NCORES):
        out[core // 4] += res.results[core]["outT"].T
    out = out + bo
    return out.reshape(B, L, F, C).astype(np.float32)
